# revision 22
# baseline (speedup 1.0000x reference)
"""Trainium2 Bass kernel for causal GQA self-attention with QK RMS-norm.

Problem (hardcoded): B=2, T=2048, d_model=2048, 16 Q heads / 4 KV heads,
head_dim=128, fp32 IO.

Sharding across 8 NeuronCores: tensor-parallel over the 4 KV head groups
(each group = 1 KV head + its 4 Q heads) x data-parallel over the 2
batches.  core = 4*b + g.  Each core computes
    qkvT_g = Wg.T @ x_b.T          ([768, T])
    q/k RMS-norm (+ per-dim scales), v transposed to natural layout
    causal attention for the 4 Q heads of group g (S^T orientation)
    yp_partial = (O^T).T @ Wp_g    ([T, d_model] partial)
and the host sums the 4 partials per batch.
"""

import functools

import numpy as np

import concourse.bass as bass
import concourse.mybir as mybir
import concourse.tile as tile
from concourse import bacc
from concourse.bass_utils import run_bass_kernel_spmd
from concourse.masks import make_identity

F32 = mybir.dt.float32
BF16 = mybir.dt.bfloat16
F32R = mybir.dt.float32r

# matmul operand dtype (stationary+moving). BF16 halves weight-load time
# (and enables FWL); fp32r is the higher-precision fallback.
MM_DT = BF16

T = 2048
C = 2048
D = 128
NH = 4            # q heads per core
NKC = C // 128    # 16 k-chunks of the d_model contraction
QKV = (NH + 2) * D  # 768 qkv rows per core
NT = 4            # 512-wide q/T tiles
TQ = 512
NEG = -1.0e30
EPS = 1e-6
SM_SCALE = 1.0 / float(np.sqrt(D))


def _pack_batches(tqt):
    """S^T j-chunk batches for q-tile `tqt`.

    Each batch is a list of (j, co, width, pos): j = key chunk, co = start
    column inside the 512-wide q tile, width = 512 - co, pos = column of
    the block inside the packed PSUM batch tile.  Blocks never cross a
    512-column PSUM bank boundary.
    """
    chunks = []
    for j in range(4 * tqt + 4):
        co = max(0, 128 * j - 512 * tqt)
        chunks.append((j, co, 512 - co))
    batches = []
    cur, pos = [], 0
    for (j, co, w) in chunks:
        if pos // 512 != (pos + w - 1) // 512 and pos % 512 != 0:
            pos = (pos // 512 + 1) * 512
        if pos + w > 1024:
            batches.append(cur)
            cur, pos = [], 0
        cur.append((j, co, w, pos))
        pos += w
    if cur:
        batches.append(cur)
    return batches


def build_kernel():
    nc = bacc.Bacc()
    xT_d = nc.dram_tensor("xT", [C, T], MM_DT, kind="ExternalInput")
    wg_d = nc.dram_tensor("wg", [C, QKV], MM_DT, kind="ExternalInput")
    wp_d = nc.dram_tensor("wp", [NH * D, C], MM_DT, kind="ExternalInput")
    qs_d = nc.dram_tensor("qs", [D, 1], F32, kind="ExternalInput")
    ks_d = nc.dram_tensor("ks", [D, 1], F32, kind="ExternalInput")
    out_d = nc.dram_tensor("out", [T, C], F32, kind="ExternalOutput")

    xT_r = xT_d.rearrange("(kc p) t -> p kc t", p=128)
    wg_r = wg_d.rearrange("(kc p) q -> p kc q", p=128)

    with tile.TileContext(nc) as tc:
        with (
            tc.tile_pool(name="consts", bufs=1) as consts,
            tc.tile_pool(name="qkv_sb", bufs=1) as qkv_sb,
        ):
            # ---- constants ----
            ident = consts.tile([128, 128], MM_DT)
            make_identity(nc, ident)
            ones32 = consts.tile([128, 128], F32)
            nc.vector.memset(ones32, 1.0)
            ones_m = consts.tile([128, 128], MM_DT)
            nc.vector.tensor_copy(ones_m, ones32)
            eps_t = consts.tile([128, 1], F32)
            nc.vector.memset(eps_t, EPS)
            qs_t = consts.tile([128, 1], F32)
            ks_t = consts.tile([128, 1], F32)
            nc.sync.dma_start(out=qs_t, in_=qs_d[:, :])
            nc.sync.dma_start(out=ks_t, in_=ks_d[:, :])
            # warm the ln/exp activation table set during the initial DMA
            # wait; Ln/Exp/Square below all live in this one set, so no
            # mid-kernel ACT_TABLE_LOAD swaps.
            warm = consts.tile([128, 1], F32)
            nc.scalar.activation(
                out=warm, in_=eps_t, func=mybir.ActivationFunctionType.Exp
            )
            nc.scalar.activation(
                out=warm, in_=eps_t, func=mybir.ActivationFunctionType.Ln
            )

            # ---- persistent activations ----
            qT = qkv_sb.tile([128, NH, T], MM_DT)   # normed q, [d, h, t]
            kT = qkv_sb.tile([128, T], MM_DT)       # normed k, [d, t]
            v_sb = qkv_sb.tile([128, NKC, 128], MM_DT)  # natural v, [tk, j, d]

            # ================= Stage A: qkvT = Wg.T @ xT =================
            with (
                tc.tile_pool(name="wg_pool", bufs=1) as wg_pool,
                tc.tile_pool(name="xt_pool", bufs=2) as xt_pool,
                tc.tile_pool(name="normtmp", bufs=4) as normtmp,
                tc.tile_pool(name="vtmp", bufs=2) as vtmp,
                tc.tile_pool(name="psA", bufs=1, space="PSUM") as psA,
                tc.tile_pool(name="psN", bufs=1, space="PSUM") as psN,
                tc.tile_pool(name="psV", bufs=1, space="PSUM") as psV,
            ):
                # per-chunk loads so the first matmuls start early
                wg_sb = wg_pool.tile([128, NKC, QKV], MM_DT)
                xts = []
                for n in range(NT):
                    xts.append(
                        xt_pool.tile(
                            [128, NKC, TQ], MM_DT, tag="xt", name=f"xt{n}"
                        )
                    )
                for kc in range(NKC):
                    nc.sync.dma_start(
                        out=xts[0][:, kc, :], in_=xT_r[:, kc, 0:TQ]
                    )
                    nc.sync.dma_start(
                        out=wg_sb[:, kc, :], in_=wg_r[:, kc, :]
                    )
                for n in range(NT):
                    xt_sb = xts[n]
                    if n + 1 < NT:
                        for g4 in range(4):
                            nc.sync.dma_start(
                                out=xts[n + 1][:, 4 * g4:4 * g4 + 4, :],
                                in_=xT_r[
                                    :, 4 * g4:4 * g4 + 4,
                                    (n + 1) * TQ:(n + 2) * TQ
                                ],
                            )
                    # kc-outer: stream chunks into 6 parallel accumulators
                    accs = [
                        psA.tile([128, TQ], F32, tag=f"acc{m}", name=f"acc{m}")
                        for m in range(6)
                    ]
                    for kc in range(NKC):
                        for m in range(6):
                            nc.tensor.matmul(
                                accs[m],
                                lhsT=wg_sb[:, kc, m * 128:(m + 1) * 128],
                                rhs=xt_sb[:, kc, :],
                                start=(kc == 0),
                                stop=(kc == NKC - 1),
                            )
                    for m in range(6):
                        acc = accs[m]
                        if m < 5:
                            # rms over partition dim via ones-matmul bcast
                            sq = normtmp.tile([128, TQ], MM_DT, tag="sq")
                            nc.scalar.activation(
                                out=sq, in_=acc,
                                func=mybir.ActivationFunctionType.Square,
                            )
                            ssq = psN.tile([128, TQ], F32, tag="ssq")
                            nc.tensor.matmul(ssq, lhsT=ones_m, rhs=sq)
                            # rinv = (ms+eps)^-0.5 via exp(-0.5*ln(ms+eps)):
                            # keeps everything in the ln/exp table set
                            lnm = normtmp.tile([128, TQ], F32, tag="lnm")
                            nc.scalar.activation(
                                out=lnm, in_=ssq,
                                func=mybir.ActivationFunctionType.Ln,
                                bias=eps_t, scale=1.0 / D,
                            )
                            rinv = normtmp.tile([128, TQ], F32, tag="rinv")
                            nc.scalar.activation(
                                out=rinv, in_=lnm,
                                func=mybir.ActivationFunctionType.Exp,
                                scale=-0.5,
                            )
                            rsc = normtmp.tile([128, TQ], F32, tag="rsc")
                            nc.vector.tensor_scalar_mul(
                                out=rsc, in0=rinv,
                                scalar1=qs_t if m < 4 else ks_t,
                            )
                            dst = (
                                qT[:, m, n * TQ:(n + 1) * TQ]
                                if m < 4
                                else kT[:, n * TQ:(n + 1) * TQ]
                            )
                            nc.vector.tensor_mul(dst, acc, rsc)
                        else:
                            # v block: transpose to natural [tk, d]
                            vt = vtmp.tile([128, TQ], MM_DT, tag="vt")
                            nc.vector.tensor_copy(vt, acc)
                            for jj in range(4):
                                vps = psV.tile([128, 128], MM_DT, tag="vps")
                                nc.tensor.transpose(
                                    vps, vt[:, jj * 128:(jj + 1) * 128], ident
                                )
                                nc.vector.tensor_copy(
                                    v_sb[:, n * 4 + jj, :], vps
                                )

            # ============ Attention + proj, per 512-wide q tile ============
            with (
                tc.tile_pool(name="attn_sb", bufs=1) as attn_sb,
                tc.tile_pool(name="pt_pool", bufs=3) as pt_pool,
                tc.tile_pool(name="rs_pool", bufs=3) as rs_pool,
                tc.tile_pool(name="yp_pool", bufs=2) as yp_pool,
                tc.tile_pool(name="psS", bufs=2, space="PSUM") as psS,
                tc.tile_pool(name="psO", bufs=1, space="PSUM") as psO,
                tc.tile_pool(name="psU", bufs=1, space="PSUM") as psU,
                tc.tile_pool(name="psC", bufs=2, space="PSUM") as psC,
            ):
                oT = attn_sb.tile([128, NH, T], MM_DT)  # attn out^T [d, h, t]
                wp_sb = attn_sb.tile([128, NH, C], MM_DT)
                wp_r = wp_d.rearrange("(h p) c -> p h c", p=128)
                for h in range(NH):
                    nc.sync.dma_start(out=wp_sb[:, h, :], in_=wp_r[:, h, :])

                for tqt in range(NT):
                    batches = _pack_batches(tqt)
                    jmax = 4 * tqt + 3

                    def flush(item):
                        # PV + row-sum matmuls for a finished exp batch;
                        # normalize the head after its last batch.
                        batch, p_sb, o_ps, u_ps, h, is_last = item
                        for (j, co, w, pos) in batch:
                            nc.tensor.matmul(
                                o_ps[:, co:TQ],
                                lhsT=v_sb[:, j, :],
                                rhs=p_sb[:, pos:pos + w],
                                start=(j == 0),
                                stop=(j == jmax),
                            )
                            nc.tensor.matmul(
                                u_ps[:, co:TQ],
                                lhsT=ones_m,
                                rhs=p_sb[:, pos:pos + w],
                                start=(j == 0),
                                stop=(j == jmax),
                            )
                        if is_last:
                            rsum = rs_pool.tile(
                                [128, TQ], F32, tag="rsum", name="rsum"
                            )
                            nc.vector.reciprocal_approx_fast(
                                out=rsum, in_=u_ps
                            )
                            nc.vector.tensor_mul(
                                oT[:, h, tqt * TQ:(tqt + 1) * TQ], o_ps, rsum
                            )

                    pending = None
                    for h in range(NH):
                        o_ps = psO.tile([128, TQ], F32, tag="o", name="o_ps")
                        u_ps = psU.tile([128, TQ], F32, tag="u", name="u_ps")
                        for bi, batch in enumerate(batches):
                            bw = batch[-1][3] + batch[-1][2]
                            s_ps = psS.tile(
                                [128, 1024], F32, tag="s", name="s_ps"
                            )
                            for (j, co, w, pos) in batch:
                                nc.tensor.matmul(
                                    s_ps[:, pos:pos + w],
                                    lhsT=kT[:, j * 128:(j + 1) * 128],
                                    rhs=qT[:, h, tqt * TQ + co:(tqt + 1) * TQ],
                                )
                            p_sb = pt_pool.tile(
                                [128, 1024], MM_DT, tag="p", name="p_sb"
                            )
                            nc.scalar.activation(
                                out=p_sb[:, 0:bw], in_=s_ps[:, 0:bw],
                                func=mybir.ActivationFunctionType.Exp,
                                scale=SM_SCALE,
                            )
                            for (j, co, w, pos) in batch:
                                if j >= 4 * tqt:
                                    # causal mask: zero p where col < row
                                    # (gpsimd: otherwise-idle engine)
                                    nc.gpsimd.affine_select(
                                        out=p_sb[:, pos:pos + 128],
                                        in_=p_sb[:, pos:pos + 128],
                                        pattern=[[1, 128]],
                                        channel_multiplier=-1, base=0,
                                        compare_op=mybir.AluOpType.is_ge,
                                        fill=0.0,
                                    )
                            if pending is not None:
                                flush(pending)
                            pending = (
                                batch, p_sb, o_ps, u_ps, h,
                                bi == len(batches) - 1,
                            )
                    if pending is not None:
                        flush(pending)
                    # ---- proj for this q tile ----
                    for tt in range(4):
                        c0 = tqt * TQ + tt * 128
                        yp = yp_pool.tile([128, C], F32, tag="yp")
                        for cn in range(4):
                            pc = psC.tile([128, TQ], F32, tag="pc")
                            for h in range(NH):
                                nc.tensor.matmul(
                                    pc,
                                    lhsT=oT[:, h, c0:c0 + 128],
                                    rhs=wp_sb[:, h, cn * TQ:(cn + 1) * TQ],
                                    start=(h == 0),
                                    stop=(h == NH - 1),
                                )
                            if cn % 2 == 0:
                                nc.scalar.copy(yp[:, cn * TQ:(cn + 1) * TQ], pc)
                            else:
                                nc.vector.tensor_copy(
                                    yp[:, cn * TQ:(cn + 1) * TQ], pc
                                )
                        nc.sync.dma_start(out=out_d[c0:c0 + 128, :], in_=yp)

    nc.finalize()
    return nc


@functools.lru_cache(maxsize=1)
def _get_nc():
    return build_kernel()


def make_in_maps(x, W_qkv, W_proj, q_scale, k_scale):
    x = np.asarray(x, dtype=np.float32)
    W_qkv = np.asarray(W_qkv, dtype=np.float32)
    W_proj = np.asarray(W_proj, dtype=np.float32)
    q_scale = np.asarray(q_scale, dtype=np.float32)
    k_scale = np.asarray(k_scale, dtype=np.float32)

    import ml_dtypes

    bf16 = ml_dtypes.bfloat16
    qs = np.ascontiguousarray(q_scale.reshape(D, 1))
    ks = np.ascontiguousarray(k_scale.reshape(D, 1))
    xT_by_batch = [np.ascontiguousarray(x[b].T).astype(bf16) for b in range(2)]
    in_maps = []
    for core in range(8):
        b, g = divmod(core, 4)
        wg = np.ascontiguousarray(
            np.concatenate(
                [
                    W_qkv[:, 512 * g:512 * (g + 1)],
                    W_qkv[:, 2048 + 128 * g:2048 + 128 * (g + 1)],
                    W_qkv[:, 2560 + 128 * g:2560 + 128 * (g + 1)],
                ],
                axis=1,
            )
        ).astype(bf16)
        wp = np.ascontiguousarray(W_proj[512 * g:512 * (g + 1), :]).astype(bf16)
        in_maps.append(
            {"xT": xT_by_batch[b], "wg": wg, "wp": wp, "qs": qs, "ks": ks}
        )
    return in_maps


def kernel(x, W_qkv, W_proj, q_scale, k_scale):
    nc = _get_nc()
    in_maps = make_in_maps(x, W_qkv, W_proj, q_scale, k_scale)
    res = run_bass_kernel_spmd(nc, in_maps, core_ids=list(range(8)))
    outs = [r["out"] for r in res.results]
    y0 = outs[0] + outs[1] + outs[2] + outs[3]
    y1 = outs[4] + outs[5] + outs[6] + outs[7]
    return np.stack([y0, y1], axis=0).astype(np.float32)


# revision 24
# speedup vs baseline: 1.1762x; 1.1762x over previous
"""Trainium2 Bass kernel for causal GQA self-attention with QK RMS-norm.

Problem (hardcoded): B=2, T=2048, d_model=2048, 16 Q heads / 4 KV heads,
head_dim=128, fp32 IO.

Sharding across 8 NeuronCores: tensor-parallel over the 4 KV head groups
(each group = 1 KV head + its 4 Q heads) x data-parallel over the 2
batches.  core = 4*b + g.  Each core computes
    qkvT_g = Wg.T @ x_b.T          ([768, T])
    q/k RMS-norm (+ per-dim scales), v transposed to natural layout
    causal attention for the 4 Q heads of group g (S^T orientation)
    yp_partial = (O^T).T @ Wp_g    ([T, d_model] partial)
and the host sums the 4 partials per batch.
"""

import functools

import numpy as np

import concourse.bass as bass
import concourse.mybir as mybir
import concourse.tile as tile
from concourse import bacc
from concourse.bass_utils import run_bass_kernel_spmd
from concourse.masks import make_identity

F32 = mybir.dt.float32
BF16 = mybir.dt.bfloat16
F32R = mybir.dt.float32r

# matmul operand dtype (stationary+moving). BF16 halves weight-load time
# (and enables FWL); fp32r is the higher-precision fallback.
MM_DT = BF16

T = 2048
C = 2048
D = 128
NH = 4            # q heads per core
NKC = C // 128    # 16 k-chunks of the d_model contraction
QKV = (NH + 2) * D  # 768 qkv rows per core
NT = 4            # 512-wide q/T tiles
TQ = 512
NEG = -1.0e30
EPS = 1e-6
SM_SCALE = 1.0 / float(np.sqrt(D))


def _pack_batches(tqt):
    """S^T j-chunk batches for q-tile `tqt`.

    Each batch is a list of (j, co, width, pos): j = key chunk, co = start
    column inside the 512-wide q tile, width = 512 - co, pos = column of
    the block inside the packed PSUM batch tile.  Blocks never cross a
    512-column PSUM bank boundary.
    """
    chunks = []
    for j in range(4 * tqt + 4):
        co = max(0, 128 * j - 512 * tqt)
        chunks.append((j, co, 512 - co))
    batches = []
    cur, pos = [], 0
    for (j, co, w) in chunks:
        if pos // 512 != (pos + w - 1) // 512 and pos % 512 != 0:
            pos = (pos // 512 + 1) * 512
        if pos + w > 1024:
            batches.append(cur)
            cur, pos = [], 0
        cur.append((j, co, w, pos))
        pos += w
    if cur:
        batches.append(cur)
    return batches


def build_kernel():
    nc = bacc.Bacc()
    xT_d = nc.dram_tensor("xT", [C, T], MM_DT, kind="ExternalInput")
    wg_d = nc.dram_tensor("wg", [C, QKV], MM_DT, kind="ExternalInput")
    wp_d = nc.dram_tensor("wp", [NH * D, C], MM_DT, kind="ExternalInput")
    qs_d = nc.dram_tensor("qs", [D, 1], F32, kind="ExternalInput")
    ks_d = nc.dram_tensor("ks", [D, 1], F32, kind="ExternalInput")
    out_d = nc.dram_tensor("out", [T, C], F32, kind="ExternalOutput")

    xT_r = xT_d.rearrange("(kc p) t -> p kc t", p=128)
    wg_r = wg_d.rearrange("(kc p) q -> p kc q", p=128)

    with tile.TileContext(nc) as tc:
        with (
            tc.tile_pool(name="consts", bufs=1) as consts,
            tc.tile_pool(name="qkv_sb", bufs=1) as qkv_sb,
        ):
            # ---- constants ----
            ident = consts.tile([128, 128], MM_DT)
            make_identity(nc, ident)
            ones32 = consts.tile([128, 128], F32)
            nc.vector.memset(ones32, 1.0)
            ones_m = consts.tile([128, 128], MM_DT)
            nc.vector.tensor_copy(ones_m, ones32)
            eps_t = consts.tile([128, 1], F32)
            nc.vector.memset(eps_t, EPS)
            qs_t = consts.tile([128, 1], F32)
            ks_t = consts.tile([128, 1], F32)
            nc.sync.dma_start(out=qs_t, in_=qs_d[:, :])
            nc.sync.dma_start(out=ks_t, in_=ks_d[:, :])

            # ---- persistent activations ----
            qT = qkv_sb.tile([128, NH, T], MM_DT)   # normed q, [d, h, t]
            kT = qkv_sb.tile([128, T], MM_DT)       # normed k, [d, t]
            v_sb = qkv_sb.tile([128, NKC, 128], MM_DT)  # natural v, [tk, j, d]

            # ================= Stage A: qkvT = Wg.T @ xT =================
            with (
                tc.tile_pool(name="wg_pool", bufs=1) as wg_pool,
                tc.tile_pool(name="xt_pool", bufs=2) as xt_pool,
                tc.tile_pool(name="normtmp", bufs=4) as normtmp,
                tc.tile_pool(name="vtmp", bufs=2) as vtmp,
                tc.tile_pool(name="psA", bufs=1, space="PSUM") as psA,
                tc.tile_pool(name="psN", bufs=1, space="PSUM") as psN,
                tc.tile_pool(name="psV", bufs=1, space="PSUM") as psV,
            ):
                # per-chunk loads so the first matmuls start early
                wg_sb = wg_pool.tile([128, NKC, QKV], MM_DT)
                xts = []
                for n in range(NT):
                    xts.append(
                        xt_pool.tile(
                            [128, NKC, TQ], MM_DT, tag="xt", name=f"xt{n}"
                        )
                    )
                for kc in range(NKC):
                    nc.sync.dma_start(
                        out=xts[0][:, kc, :], in_=xT_r[:, kc, 0:TQ]
                    )
                    nc.sync.dma_start(
                        out=wg_sb[:, kc, :], in_=wg_r[:, kc, :]
                    )
                for n in range(NT):
                    xt_sb = xts[n]
                    if n + 1 < NT:
                        for g4 in range(4):
                            nc.sync.dma_start(
                                out=xts[n + 1][:, 4 * g4:4 * g4 + 4, :],
                                in_=xT_r[
                                    :, 4 * g4:4 * g4 + 4,
                                    (n + 1) * TQ:(n + 2) * TQ
                                ],
                            )
                    # kc-outer: stream chunks into 6 parallel accumulators
                    accs = [
                        psA.tile([128, TQ], F32, tag=f"acc{m}", name=f"acc{m}")
                        for m in range(6)
                    ]
                    for kc in range(NKC):
                        for m in range(6):
                            nc.tensor.matmul(
                                accs[m],
                                lhsT=wg_sb[:, kc, m * 128:(m + 1) * 128],
                                rhs=xt_sb[:, kc, :],
                                start=(kc == 0),
                                stop=(kc == NKC - 1),
                            )
                    for m in range(6):
                        acc = accs[m]
                        if m < 5:
                            # rms over partition dim via ones-matmul bcast
                            sq = normtmp.tile([128, TQ], MM_DT, tag="sq")
                            nc.scalar.activation(
                                out=sq, in_=acc,
                                func=mybir.ActivationFunctionType.Square,
                            )
                            ssq = psN.tile([128, TQ], F32, tag="ssq")
                            nc.tensor.matmul(ssq, lhsT=ones_m, rhs=sq)
                            rms = normtmp.tile([128, TQ], F32, tag="rms")
                            nc.scalar.activation(
                                out=rms, in_=ssq,
                                func=mybir.ActivationFunctionType.Sqrt,
                                bias=eps_t, scale=1.0 / D,
                            )
                            rinv = normtmp.tile([128, TQ], F32, tag="rinv")
                            nc.vector.reciprocal_approx_fast(out=rinv, in_=rms)
                            rsc = normtmp.tile([128, TQ], F32, tag="rsc")
                            nc.vector.tensor_scalar_mul(
                                out=rsc, in0=rinv,
                                scalar1=qs_t if m < 4 else ks_t,
                            )
                            dst = (
                                qT[:, m, n * TQ:(n + 1) * TQ]
                                if m < 4
                                else kT[:, n * TQ:(n + 1) * TQ]
                            )
                            nc.vector.tensor_mul(dst, acc, rsc)
                        else:
                            # v block: transpose to natural [tk, d]
                            vt = vtmp.tile([128, TQ], MM_DT, tag="vt")
                            nc.vector.tensor_copy(vt, acc)
                            for jj in range(4):
                                vps = psV.tile([128, 128], MM_DT, tag="vps")
                                nc.tensor.transpose(
                                    vps, vt[:, jj * 128:(jj + 1) * 128], ident
                                )
                                nc.vector.tensor_copy(
                                    v_sb[:, n * 4 + jj, :], vps
                                )

            # ============ Attention + proj, per 512-wide q tile ============
            with (
                tc.tile_pool(name="attn_sb", bufs=1) as attn_sb,
                tc.tile_pool(name="pt_pool", bufs=3) as pt_pool,
                tc.tile_pool(name="rs_pool", bufs=3) as rs_pool,
                tc.tile_pool(name="yp_pool", bufs=2) as yp_pool,
                tc.tile_pool(name="psS", bufs=2, space="PSUM") as psS,
                tc.tile_pool(name="psO", bufs=1, space="PSUM") as psO,
                tc.tile_pool(name="psU", bufs=1, space="PSUM") as psU,
                tc.tile_pool(name="psC", bufs=2, space="PSUM") as psC,
            ):
                oT = attn_sb.tile([128, NH, T], MM_DT)  # attn out^T [d, h, t]
                wp_sb = attn_sb.tile([128, NH, C], MM_DT)
                wp_r = wp_d.rearrange("(h p) c -> p h c", p=128)
                for h in range(NH):
                    nc.sync.dma_start(out=wp_sb[:, h, :], in_=wp_r[:, h, :])

                for tqt in range(NT):
                    batches = _pack_batches(tqt)
                    jmax = 4 * tqt + 3

                    def flush(item):
                        # PV + row-sum matmuls for a finished exp batch;
                        # normalize the head after its last batch.
                        batch, p_sb, o_ps, u_ps, h, is_last = item
                        for (j, co, w, pos) in batch:
                            nc.tensor.matmul(
                                o_ps[:, co:TQ],
                                lhsT=v_sb[:, j, :],
                                rhs=p_sb[:, pos:pos + w],
                                start=(j == 0),
                                stop=(j == jmax),
                            )
                            nc.tensor.matmul(
                                u_ps[:, co:TQ],
                                lhsT=ones_m,
                                rhs=p_sb[:, pos:pos + w],
                                start=(j == 0),
                                stop=(j == jmax),
                            )
                        if is_last:
                            rsum = rs_pool.tile(
                                [128, TQ], F32, tag="rsum", name="rsum"
                            )
                            nc.vector.reciprocal_approx_fast(
                                out=rsum, in_=u_ps
                            )
                            nc.vector.tensor_mul(
                                oT[:, h, tqt * TQ:(tqt + 1) * TQ], o_ps, rsum
                            )

                    pending = None
                    for h in range(NH):
                        o_ps = psO.tile([128, TQ], F32, tag="o", name="o_ps")
                        u_ps = psU.tile([128, TQ], F32, tag="u", name="u_ps")
                        for bi, batch in enumerate(batches):
                            bw = batch[-1][3] + batch[-1][2]
                            s_ps = psS.tile(
                                [128, 1024], F32, tag="s", name="s_ps"
                            )
                            for (j, co, w, pos) in batch:
                                nc.tensor.matmul(
                                    s_ps[:, pos:pos + w],
                                    lhsT=kT[:, j * 128:(j + 1) * 128],
                                    rhs=qT[:, h, tqt * TQ + co:(tqt + 1) * TQ],
                                )
                            p_sb = pt_pool.tile(
                                [128, 1024], MM_DT, tag="p", name="p_sb"
                            )
                            nc.scalar.activation(
                                out=p_sb[:, 0:bw], in_=s_ps[:, 0:bw],
                                func=mybir.ActivationFunctionType.Exp,
                                scale=SM_SCALE,
                            )
                            for (j, co, w, pos) in batch:
                                if j >= 4 * tqt:
                                    # causal mask: zero p where col < row
                                    # (gpsimd: otherwise-idle engine)
                                    nc.gpsimd.affine_select(
                                        out=p_sb[:, pos:pos + 128],
                                        in_=p_sb[:, pos:pos + 128],
                                        pattern=[[1, 128]],
                                        channel_multiplier=-1, base=0,
                                        compare_op=mybir.AluOpType.is_ge,
                                        fill=0.0,
                                    )
                            if pending is not None:
                                flush(pending)
                            pending = (
                                batch, p_sb, o_ps, u_ps, h,
                                bi == len(batches) - 1,
                            )
                    if pending is not None:
                        flush(pending)
                    # ---- proj for this q tile ----
                    for tt in range(4):
                        c0 = tqt * TQ + tt * 128
                        yp = yp_pool.tile([128, C], F32, tag="yp")
                        for cn in range(4):
                            pc = psC.tile([128, TQ], F32, tag="pc")
                            for h in range(NH):
                                nc.tensor.matmul(
                                    pc,
                                    lhsT=oT[:, h, c0:c0 + 128],
                                    rhs=wp_sb[:, h, cn * TQ:(cn + 1) * TQ],
                                    start=(h == 0),
                                    stop=(h == NH - 1),
                                )
                            if cn % 2 == 0:
                                nc.scalar.copy(yp[:, cn * TQ:(cn + 1) * TQ], pc)
                            else:
                                nc.vector.tensor_copy(
                                    yp[:, cn * TQ:(cn + 1) * TQ], pc
                                )
                        nc.sync.dma_start(out=out_d[c0:c0 + 128, :], in_=yp)

    nc.finalize()
    return nc


@functools.lru_cache(maxsize=1)
def _get_nc():
    return build_kernel()


def make_in_maps(x, W_qkv, W_proj, q_scale, k_scale):
    x = np.asarray(x, dtype=np.float32)
    W_qkv = np.asarray(W_qkv, dtype=np.float32)
    W_proj = np.asarray(W_proj, dtype=np.float32)
    q_scale = np.asarray(q_scale, dtype=np.float32)
    k_scale = np.asarray(k_scale, dtype=np.float32)

    import ml_dtypes

    bf16 = ml_dtypes.bfloat16
    qs = np.ascontiguousarray(q_scale.reshape(D, 1))
    ks = np.ascontiguousarray(k_scale.reshape(D, 1))
    xT_by_batch = [np.ascontiguousarray(x[b].T).astype(bf16) for b in range(2)]
    in_maps = []
    for core in range(8):
        b, g = divmod(core, 4)
        wg = np.ascontiguousarray(
            np.concatenate(
                [
                    W_qkv[:, 512 * g:512 * (g + 1)],
                    W_qkv[:, 2048 + 128 * g:2048 + 128 * (g + 1)],
                    W_qkv[:, 2560 + 128 * g:2560 + 128 * (g + 1)],
                ],
                axis=1,
            )
        ).astype(bf16)
        wp = np.ascontiguousarray(W_proj[512 * g:512 * (g + 1), :]).astype(bf16)
        in_maps.append(
            {"xT": xT_by_batch[b], "wg": wg, "wp": wp, "qs": qs, "ks": ks}
        )
    return in_maps


def kernel(x, W_qkv, W_proj, q_scale, k_scale):
    nc = _get_nc()
    in_maps = make_in_maps(x, W_qkv, W_proj, q_scale, k_scale)
    res = run_bass_kernel_spmd(nc, in_maps, core_ids=list(range(8)))
    outs = [r["out"] for r in res.results]
    y0 = outs[0] + outs[1] + outs[2] + outs[3]
    y1 = outs[4] + outs[5] + outs[6] + outs[7]
    return np.stack([y0, y1], axis=0).astype(np.float32)


# revision 25
# speedup vs baseline: 1.1882x; 1.0102x over previous
"""Trainium2 Bass kernel for causal GQA self-attention with QK RMS-norm.

Problem (hardcoded): B=2, T=2048, d_model=2048, 16 Q heads / 4 KV heads,
head_dim=128, fp32 IO.

Sharding across 8 NeuronCores: tensor-parallel over the 4 KV head groups
(each group = 1 KV head + its 4 Q heads) x data-parallel over the 2
batches.  core = 4*b + g.  Each core computes
    qkvT_g = Wg.T @ x_b.T          ([768, T])
    q/k RMS-norm (+ per-dim scales), v transposed to natural layout
    causal attention for the 4 Q heads of group g (S^T orientation)
    yp_partial = (O^T).T @ Wp_g    ([T, d_model] partial)
and the host sums the 4 partials per batch.
"""

import functools

import numpy as np

import concourse.bass as bass
import concourse.mybir as mybir
import concourse.tile as tile
from concourse import bacc
from concourse.bass_utils import run_bass_kernel_spmd
from concourse.masks import make_identity

F32 = mybir.dt.float32
BF16 = mybir.dt.bfloat16
F32R = mybir.dt.float32r

# matmul operand dtype (stationary+moving). BF16 halves weight-load time
# (and enables FWL); fp32r is the higher-precision fallback.
MM_DT = BF16

T = 2048
C = 2048
D = 128
NH = 4            # q heads per core
NKC = C // 128    # 16 k-chunks of the d_model contraction
QKV = (NH + 2) * D  # 768 qkv rows per core
NT = 4            # 512-wide q/T tiles
TQ = 512
NEG = -1.0e30
EPS = 1e-6
SM_SCALE = 1.0 / float(np.sqrt(D))


def _pack_batches(tqt):
    """S^T j-chunk batches for q-tile `tqt`.

    Each batch is a list of (j, co, width, pos): j = key chunk, co = start
    column inside the 512-wide q tile, width = 512 - co, pos = column of
    the block inside the packed PSUM batch tile.  Blocks never cross a
    512-column PSUM bank boundary.
    """
    chunks = []
    for j in range(4 * tqt + 4):
        co = max(0, 128 * j - 512 * tqt)
        chunks.append((j, co, 512 - co))
    batches = []
    cur, pos = [], 0
    for (j, co, w) in chunks:
        if pos // 512 != (pos + w - 1) // 512 and pos % 512 != 0:
            pos = (pos // 512 + 1) * 512
        if pos + w > 1024:
            batches.append(cur)
            cur, pos = [], 0
        cur.append((j, co, w, pos))
        pos += w
    if cur:
        batches.append(cur)
    return batches


def build_kernel():
    nc = bacc.Bacc()
    xT_d = nc.dram_tensor("xT", [C, T], MM_DT, kind="ExternalInput")
    wg_d = nc.dram_tensor("wg", [C, QKV], MM_DT, kind="ExternalInput")
    wp_d = nc.dram_tensor("wp", [NH * D, C], MM_DT, kind="ExternalInput")
    qs_d = nc.dram_tensor("qs", [D, 1], F32, kind="ExternalInput")
    ks_d = nc.dram_tensor("ks", [D, 1], F32, kind="ExternalInput")
    out_d = nc.dram_tensor("out", [T, C], F32, kind="ExternalOutput")

    xT_r = xT_d.rearrange("(kc p) t -> p kc t", p=128)
    wg_r = wg_d.rearrange("(kc p) q -> p kc q", p=128)

    with tile.TileContext(nc) as tc:
        with (
            tc.tile_pool(name="consts", bufs=1) as consts,
            tc.tile_pool(name="qkv_sb", bufs=1) as qkv_sb,
        ):
            # ---- constants ----
            ident = consts.tile([128, 128], MM_DT)
            make_identity(nc, ident)
            ones32 = consts.tile([128, 128], F32)
            nc.vector.memset(ones32, 1.0)
            ones_m = consts.tile([128, 128], MM_DT)
            nc.vector.tensor_copy(ones_m, ones32)
            eps_t = consts.tile([128, 1], F32)
            nc.vector.memset(eps_t, EPS)
            qs_t = consts.tile([128, 1], F32)
            ks_t = consts.tile([128, 1], F32)
            nc.sync.dma_start(out=qs_t, in_=qs_d[:, :])
            nc.sync.dma_start(out=ks_t, in_=ks_d[:, :])

            # ---- persistent activations ----
            qT = qkv_sb.tile([128, NH, T], MM_DT)   # normed q, [d, h, t]
            kT = qkv_sb.tile([128, T], MM_DT)       # normed k, [d, t]
            v_sb = qkv_sb.tile([128, NKC, 128], MM_DT)  # natural v, [tk, j, d]

            # ================= Stage A: qkvT = Wg.T @ xT =================
            with (
                tc.tile_pool(name="wg_pool", bufs=1) as wg_pool,
                tc.tile_pool(name="xt_pool", bufs=2) as xt_pool,
                tc.tile_pool(name="normtmp", bufs=4) as normtmp,
                tc.tile_pool(name="vtmp", bufs=2) as vtmp,
                tc.tile_pool(name="psA", bufs=1, space="PSUM") as psA,
                tc.tile_pool(name="psN", bufs=1, space="PSUM") as psN,
                tc.tile_pool(name="psV", bufs=1, space="PSUM") as psV,
            ):
                # per-chunk loads so the first matmuls start early
                wg_sb = wg_pool.tile([128, NKC, QKV], MM_DT)
                xts = []
                for n in range(NT):
                    xts.append(
                        xt_pool.tile(
                            [128, NKC, TQ], MM_DT, tag="xt", name=f"xt{n}"
                        )
                    )
                for kc in range(NKC):
                    nc.sync.dma_start(
                        out=xts[0][:, kc, :], in_=xT_r[:, kc, 0:TQ]
                    )
                    nc.sync.dma_start(
                        out=wg_sb[:, kc, :], in_=wg_r[:, kc, :]
                    )
                for n in range(NT):
                    xt_sb = xts[n]
                    if n + 1 < NT:
                        for g4 in range(4):
                            nc.sync.dma_start(
                                out=xts[n + 1][:, 4 * g4:4 * g4 + 4, :],
                                in_=xT_r[
                                    :, 4 * g4:4 * g4 + 4,
                                    (n + 1) * TQ:(n + 2) * TQ
                                ],
                            )
                    # kc-outer: stream chunks into 6 parallel accumulators
                    accs = [
                        psA.tile([128, TQ], F32, tag=f"acc{m}", name=f"acc{m}")
                        for m in range(6)
                    ]
                    for kc in range(NKC):
                        for m in range(6):
                            nc.tensor.matmul(
                                accs[m],
                                lhsT=wg_sb[:, kc, m * 128:(m + 1) * 128],
                                rhs=xt_sb[:, kc, :],
                                start=(kc == 0),
                                stop=(kc == NKC - 1),
                            )
                    for m in range(6):
                        acc = accs[m]
                        if m < 5:
                            # rms over partition dim via ones-matmul bcast
                            sq = normtmp.tile([128, TQ], MM_DT, tag="sq")
                            nc.scalar.activation(
                                out=sq, in_=acc,
                                func=mybir.ActivationFunctionType.Square,
                            )
                            ssq = psN.tile([128, TQ], F32, tag="ssq")
                            nc.tensor.matmul(ssq, lhsT=ones_m, rhs=sq)
                            rms = normtmp.tile([128, TQ], F32, tag="rms")
                            nc.scalar.activation(
                                out=rms, in_=ssq,
                                func=mybir.ActivationFunctionType.Sqrt,
                                bias=eps_t, scale=1.0 / D,
                            )
                            rinv = normtmp.tile([128, TQ], F32, tag="rinv")
                            nc.vector.reciprocal_approx_fast(out=rinv, in_=rms)
                            rsc = normtmp.tile([128, TQ], F32, tag="rsc")
                            nc.vector.tensor_scalar_mul(
                                out=rsc, in0=rinv,
                                scalar1=qs_t if m < 4 else ks_t,
                            )
                            dst = (
                                qT[:, m, n * TQ:(n + 1) * TQ]
                                if m < 4
                                else kT[:, n * TQ:(n + 1) * TQ]
                            )
                            nc.vector.tensor_mul(dst, acc, rsc)
                        else:
                            # v block: transpose to natural [tk, d]
                            vt = vtmp.tile([128, TQ], MM_DT, tag="vt")
                            nc.vector.tensor_copy(vt, acc)
                            for jj in range(4):
                                vps = psV.tile([128, 128], MM_DT, tag="vps")
                                nc.tensor.transpose(
                                    vps, vt[:, jj * 128:(jj + 1) * 128], ident
                                )
                                nc.vector.tensor_copy(
                                    v_sb[:, n * 4 + jj, :], vps
                                )

            # ============ Attention + proj, per 512-wide q tile ============
            with (
                tc.tile_pool(name="attn_sb", bufs=1) as attn_sb,
                tc.tile_pool(name="pt_pool", bufs=3) as pt_pool,
                tc.tile_pool(name="rs_pool", bufs=3) as rs_pool,
                tc.tile_pool(name="yp_pool", bufs=2) as yp_pool,
                tc.tile_pool(name="psS", bufs=2, space="PSUM") as psS,
                tc.tile_pool(name="psO", bufs=1, space="PSUM") as psO,
                tc.tile_pool(name="psU", bufs=1, space="PSUM") as psU,
                tc.tile_pool(name="psC", bufs=2, space="PSUM") as psC,
            ):
                oT = attn_sb.tile([128, NH, T], MM_DT)  # attn out^T [d, h, t]
                wp_sb = attn_sb.tile([128, NH, C], MM_DT)
                wp_r = wp_d.rearrange("(h p) c -> p h c", p=128)
                for h in range(NH):
                    nc.sync.dma_start(out=wp_sb[:, h, :], in_=wp_r[:, h, :])

                def emit_proj(tqt):
                    # y[tq tile] = (oT).T @ wp for this 512-wide q tile
                    for tt in range(4):
                        c0 = tqt * TQ + tt * 128
                        yp = yp_pool.tile([128, C], F32, tag="yp", name="yp")
                        for cn in range(4):
                            pc = psC.tile([128, TQ], F32, tag="pc", name="pc")
                            for h in range(NH):
                                nc.tensor.matmul(
                                    pc,
                                    lhsT=oT[:, h, c0:c0 + 128],
                                    rhs=wp_sb[:, h, cn * TQ:(cn + 1) * TQ],
                                    start=(h == 0),
                                    stop=(h == NH - 1),
                                )
                            if cn == 0:
                                nc.scalar.copy(yp[:, cn * TQ:(cn + 1) * TQ], pc)
                            else:
                                nc.vector.tensor_copy(
                                    yp[:, cn * TQ:(cn + 1) * TQ], pc
                                )
                        nc.sync.dma_start(out=out_d[c0:c0 + 128, :], in_=yp)

                def flush(item):
                    # PV + row-sum matmuls for a finished exp batch;
                    # normalize the head after its last batch; emit proj
                    # for the q tile after its last head.
                    batch, p_sb, o_ps, u_ps, tqt, h, is_last, jmax = item
                    for (j, co, w, pos) in batch:
                        nc.tensor.matmul(
                            o_ps[:, co:TQ],
                            lhsT=v_sb[:, j, :],
                            rhs=p_sb[:, pos:pos + w],
                            start=(j == 0),
                            stop=(j == jmax),
                        )
                        nc.tensor.matmul(
                            u_ps[:, co:TQ],
                            lhsT=ones_m,
                            rhs=p_sb[:, pos:pos + w],
                            start=(j == 0),
                            stop=(j == jmax),
                        )
                    if is_last:
                        rsum = rs_pool.tile(
                            [128, TQ], F32, tag="rsum", name="rsum"
                        )
                        nc.vector.reciprocal_approx_fast(out=rsum, in_=u_ps)
                        nc.vector.tensor_mul(
                            oT[:, h, tqt * TQ:(tqt + 1) * TQ], o_ps, rsum
                        )
                        if h == NH - 1:
                            emit_proj(tqt)

                pending = None
                for tqt in range(NT):
                    batches = _pack_batches(tqt)
                    jmax = 4 * tqt + 3
                    for h in range(NH):
                        o_ps = psO.tile([128, TQ], F32, tag="o", name="o_ps")
                        u_ps = psU.tile([128, TQ], F32, tag="u", name="u_ps")
                        for bi, batch in enumerate(batches):
                            bw = batch[-1][3] + batch[-1][2]
                            s_ps = psS.tile(
                                [128, 1024], F32, tag="s", name="s_ps"
                            )
                            for (j, co, w, pos) in batch:
                                nc.tensor.matmul(
                                    s_ps[:, pos:pos + w],
                                    lhsT=kT[:, j * 128:(j + 1) * 128],
                                    rhs=qT[:, h, tqt * TQ + co:(tqt + 1) * TQ],
                                )
                            p_sb = pt_pool.tile(
                                [128, 1024], MM_DT, tag="p", name="p_sb"
                            )
                            nc.scalar.activation(
                                out=p_sb[:, 0:bw], in_=s_ps[:, 0:bw],
                                func=mybir.ActivationFunctionType.Exp,
                                scale=SM_SCALE,
                            )
                            for (j, co, w, pos) in batch:
                                if j >= 4 * tqt:
                                    # causal mask: zero p where col < row
                                    # (gpsimd: otherwise-idle engine)
                                    nc.gpsimd.affine_select(
                                        out=p_sb[:, pos:pos + 128],
                                        in_=p_sb[:, pos:pos + 128],
                                        pattern=[[1, 128]],
                                        channel_multiplier=-1, base=0,
                                        compare_op=mybir.AluOpType.is_ge,
                                        fill=0.0,
                                    )
                            if pending is not None:
                                flush(pending)
                            pending = (
                                batch, p_sb, o_ps, u_ps, tqt, h,
                                bi == len(batches) - 1, jmax,
                            )
                if pending is not None:
                    flush(pending)

    nc.finalize()
    return nc


@functools.lru_cache(maxsize=1)
def _get_nc():
    return build_kernel()


def make_in_maps(x, W_qkv, W_proj, q_scale, k_scale):
    x = np.asarray(x, dtype=np.float32)
    W_qkv = np.asarray(W_qkv, dtype=np.float32)
    W_proj = np.asarray(W_proj, dtype=np.float32)
    q_scale = np.asarray(q_scale, dtype=np.float32)
    k_scale = np.asarray(k_scale, dtype=np.float32)

    import ml_dtypes

    bf16 = ml_dtypes.bfloat16
    qs = np.ascontiguousarray(q_scale.reshape(D, 1))
    ks = np.ascontiguousarray(k_scale.reshape(D, 1))
    xT_by_batch = [np.ascontiguousarray(x[b].T).astype(bf16) for b in range(2)]
    in_maps = []
    for core in range(8):
        b, g = divmod(core, 4)
        wg = np.ascontiguousarray(
            np.concatenate(
                [
                    W_qkv[:, 512 * g:512 * (g + 1)],
                    W_qkv[:, 2048 + 128 * g:2048 + 128 * (g + 1)],
                    W_qkv[:, 2560 + 128 * g:2560 + 128 * (g + 1)],
                ],
                axis=1,
            )
        ).astype(bf16)
        wp = np.ascontiguousarray(W_proj[512 * g:512 * (g + 1), :]).astype(bf16)
        in_maps.append(
            {"xT": xT_by_batch[b], "wg": wg, "wp": wp, "qs": qs, "ks": ks}
        )
    return in_maps


def kernel(x, W_qkv, W_proj, q_scale, k_scale):
    nc = _get_nc()
    in_maps = make_in_maps(x, W_qkv, W_proj, q_scale, k_scale)
    res = run_bass_kernel_spmd(nc, in_maps, core_ids=list(range(8)))
    outs = [r["out"] for r in res.results]
    y0 = outs[0] + outs[1] + outs[2] + outs[3]
    y1 = outs[4] + outs[5] + outs[6] + outs[7]
    return np.stack([y0, y1], axis=0).astype(np.float32)
